# revision 3
# baseline (speedup 1.0000x reference)
"""Trainium2 Bass kernel for nn_LogisticModel.

Computes, elementwise over [B, T] f32 inputs s, x:
    x_prev[:, t] = x[:, t-1]  (0 for t == 0)
    bias  = sigmoid(gain * s)
    resid = x - decay * x_prev - bias
    logp  = -0.5 * (resid / noise)^2 - (log(noise) + 0.5*log(2*pi))

Data-parallel over the batch axis: each of the 8 NeuronCores processes
B/8 = 512 rows. No cross-core communication (rows are independent).

Per-core schedule (memory-bound; HBM roofline ~48 MiB / ~360 GB/s ~ 140 us):
  - tiles of [128, W] columns; x is loaded as [128, W+1] with one extra
    leading column so both x and x_prev views come from a single DMA.
  - ACT (scalar engine): sigmoid, square(scale), final affine copy.
  - DVE (vector engine): (x_prev * -decay) + x, then subtract bias.
"""

import os
import sys
from contextlib import ExitStack

import numpy as np

for _p in ("/root/.axon_site", "/root/.axon_site/_ro/trn_rl_repo",
           "/root/.axon_site/_ro/pypackages", "/opt/trn_rl_repo"):
    if os.path.isdir(_p) and _p not in sys.path:
        sys.path.append(_p)

import concourse.bass as bass
import concourse.bacc as bacc
import concourse.mybir as mybir
import concourse.tile as tile

F32 = mybir.dt.float32
P = 128

N_CORES = 8
B, T = 4096, 8192

LAST_RESULT = None  # test harness introspection; unused by graders


def build_module(rows, cols, gain, decay, noise, W=2048, bufs=3):
    """Build the single-core Bass module for a [rows, cols] shard."""
    assert rows % P == 0 and cols % W == 0
    nc = bacc.Bacc()
    s_in = nc.declare_dram_parameter("s", [rows, cols], F32, isOutput=False)
    x_in = nc.declare_dram_parameter("x", [rows, cols], F32, isOutput=False)
    out = nc.declare_dram_parameter("out", [rows, cols], F32, isOutput=True)

    log_norm = float(np.log(noise) + 0.5 * np.log(2.0 * np.pi))
    inv_noise = float(1.0 / noise)
    AF = mybir.ActivationFunctionType
    OP = mybir.AluOpType

    with tile.TileContext(nc) as tc, ExitStack() as ctx:
        loads = ctx.enter_context(tc.tile_pool(name="loads", bufs=bufs))
        mids = ctx.enter_context(tc.tile_pool(name="mids", bufs=2))
        outs = ctx.enter_context(tc.tile_pool(name="outs", bufs=bufs))
        for rb in range(rows // P):
            r0 = rb * P
            for cb in range(cols // W):
                c0 = cb * W
                s_t = loads.tile([P, W], F32, tag="s")
                nc.sync.dma_start(s_t[:], s_in[r0:r0 + P, c0:c0 + W])
                bias_t = mids.tile([P, W], F32, tag="bias")
                nc.scalar.activation(bias_t[:], s_t[:], AF.Sigmoid,
                                     scale=float(gain))
                t_t = mids.tile([P, W], F32, tag="t")
                # x tile carries one extra leading column = x_prev source.
                # t = x - decay * x_prev.  STT format (3 APs) only has room
                # for ONE sync wait, so x_t must have exactly one producer:
                # for the first column tile, load aligned and handle t=0
                # (x_prev = 0) with a 1-column copy instead of a memset.
                x_t = loads.tile([P, W + 1], F32, tag="x")
                if cb == 0:
                    nc.sync.dma_start(x_t[:, 0:W], x_in[r0:r0 + P, 0:W])
                    nc.vector.scalar_tensor_tensor(
                        t_t[:, 1:W], x_t[:, 0:W - 1], -float(decay),
                        x_t[:, 1:W], OP.mult, OP.add)
                    nc.vector.tensor_copy(t_t[:, 0:1], x_t[:, 0:1])
                else:
                    nc.sync.dma_start(x_t[:], x_in[r0:r0 + P, c0 - 1:c0 + W])
                    nc.vector.scalar_tensor_tensor(
                        t_t[:], x_t[:, 0:W], -float(decay), x_t[:, 1:W + 1],
                        OP.mult, OP.add)
                resid_t = mids.tile([P, W], F32, tag="resid")
                nc.vector.tensor_tensor(resid_t[:], t_t[:], bias_t[:],
                                        OP.subtract)
                r2_t = mids.tile([P, W], F32, tag="r2")
                nc.scalar.activation(r2_t[:], resid_t[:], AF.Square,
                                     scale=inv_noise)
                o_t = outs.tile([P, W], F32, tag="o")
                nc.scalar.activation(o_t[:], r2_t[:], AF.Copy,
                                     bias=-log_norm, scale=-0.5)
                nc.sync.dma_start(out[r0:r0 + P, c0:c0 + W], o_t[:])
    # Bacc.compile() legalizes sync waits (TRN2: max 1 wait per instruction)
    nc.compile()
    return nc


_MODULE_CACHE = {}


def _get_module(key):
    if key not in _MODULE_CACHE:
        _MODULE_CACHE[key] = build_module(*key)
    return _MODULE_CACHE[key]


def kernel(s, x, gain, decay, noise):
    global LAST_RESULT
    from concourse.bass_utils import run_bass_kernel_spmd

    s = np.ascontiguousarray(np.asarray(s, dtype=np.float32))
    x = np.ascontiguousarray(np.asarray(x, dtype=np.float32))
    b, t = s.shape
    assert b % N_CORES == 0
    rows = b // N_CORES

    nc = _get_module((rows, t, float(gain), float(decay), float(noise)))

    in_maps = [
        {"s": s[i * rows:(i + 1) * rows], "x": x[i * rows:(i + 1) * rows]}
        for i in range(N_CORES)
    ]
    res = run_bass_kernel_spmd(nc, in_maps, list(range(N_CORES)))
    LAST_RESULT = res
    return np.concatenate([res.results[i]["out"] for i in range(N_CORES)],
                          axis=0)


# revision 4
# speedup vs baseline: 1.0589x; 1.0589x over previous
"""Trainium2 Bass kernel for nn_LogisticModel.

Computes, elementwise over [B, T] f32 inputs s, x:
    x_prev[:, t] = x[:, t-1]  (0 for t == 0)
    bias  = sigmoid(gain * s)
    resid = x - decay * x_prev - bias
    logp  = -0.5 * (resid / noise)^2 - (log(noise) + 0.5*log(2*pi))

Data-parallel over the batch axis: each of the 8 NeuronCores processes
B/8 = 512 rows. No cross-core communication (rows are independent).

Per-core schedule (memory-bound; HBM roofline ~48 MiB / ~360 GB/s ~ 140 us):
  - tiles of [128, W] columns; x is loaded as [128, W+1] with one extra
    leading column so both x and x_prev views come from a single DMA.
  - ACT (scalar engine): sigmoid, square(scale), final affine copy.
  - DVE (vector engine): (x_prev * -decay) + x, then subtract bias.
"""

import os
import sys
from contextlib import ExitStack

import numpy as np

for _p in ("/root/.axon_site", "/root/.axon_site/_ro/trn_rl_repo",
           "/root/.axon_site/_ro/pypackages", "/opt/trn_rl_repo"):
    if os.path.isdir(_p) and _p not in sys.path:
        sys.path.append(_p)

import concourse.bass as bass
import concourse.bacc as bacc
import concourse.mybir as mybir
import concourse.tile as tile

F32 = mybir.dt.float32
P = 128

N_CORES = 8
B, T = 4096, 8192

LAST_RESULT = None  # test harness introspection; unused by graders


def build_module(rows, cols, gain, decay, noise, W=4096, load_bufs=4,
                 work_bufs=3):
    """Build the single-core Bass module for a [rows, cols] shard."""
    assert rows % P == 0 and cols % W == 0
    nc = bacc.Bacc()
    s_in = nc.declare_dram_parameter("s", [rows, cols], F32, isOutput=False)
    x_in = nc.declare_dram_parameter("x", [rows, cols], F32, isOutput=False)
    out = nc.declare_dram_parameter("out", [rows, cols], F32, isOutput=True)

    log_norm = float(np.log(noise) + 0.5 * np.log(2.0 * np.pi))
    inv_noise = float(1.0 / noise)
    AF = mybir.ActivationFunctionType
    OP = mybir.AluOpType

    with tile.TileContext(nc) as tc, ExitStack() as ctx:
        loads = ctx.enter_context(tc.tile_pool(name="loads", bufs=load_bufs))
        work = ctx.enter_context(tc.tile_pool(name="work", bufs=work_bufs))
        for rb in range(rows // P):
            r0 = rb * P
            for cb in range(cols // W):
                c0 = cb * W
                # Loads on the SP HWDGE ring; stores on the ACT ring so
                # output stores don't head-of-line-block upcoming loads.
                s_t = loads.tile([P, W], F32, tag="s")
                nc.sync.dma_start(s_t[:], s_in[r0:r0 + P, c0:c0 + W])
                # x tile carries one extra leading column = x_prev source.
                # STT format (3 APs) only has room for ONE sync wait, so
                # x_t must have exactly one producer: for the first column
                # tile, load aligned and handle t=0 (x_prev = 0) with a
                # 1-column copy instead of a memset.
                x_t = loads.tile([P, W + 1], F32, tag="x")
                # bias = sigmoid(gain * s), in place over s
                nc.scalar.activation(s_t[:], s_t[:], AF.Sigmoid,
                                     scale=float(gain))
                t_t = work.tile([P, W], F32, tag="t")
                # t = x - decay * x_prev
                if cb == 0:
                    nc.sync.dma_start(x_t[:, 0:W], x_in[r0:r0 + P, 0:W])
                    nc.vector.scalar_tensor_tensor(
                        t_t[:, 1:W], x_t[:, 0:W - 1], -float(decay),
                        x_t[:, 1:W], OP.mult, OP.add)
                    nc.vector.tensor_copy(t_t[:, 0:1], x_t[:, 0:1])
                else:
                    nc.sync.dma_start(x_t[:], x_in[r0:r0 + P, c0 - 1:c0 + W])
                    nc.vector.scalar_tensor_tensor(
                        t_t[:], x_t[:, 0:W], -float(decay), x_t[:, 1:W + 1],
                        OP.mult, OP.add)
                # resid = t - bias;  r2 = (resid/noise)^2;  out affine —
                # all in place over t_t.
                nc.vector.tensor_tensor(t_t[:], t_t[:], s_t[:], OP.subtract)
                nc.scalar.activation(t_t[:], t_t[:], AF.Square,
                                     scale=inv_noise)
                nc.scalar.activation(t_t[:], t_t[:], AF.Copy,
                                     bias=-log_norm, scale=-0.5)
                nc.scalar.dma_start(out[r0:r0 + P, c0:c0 + W], t_t[:])
    # Bacc.compile() legalizes sync waits (TRN2: max 1 wait per instruction)
    nc.compile()
    return nc


_MODULE_CACHE = {}


def _get_module(key):
    if key not in _MODULE_CACHE:
        _MODULE_CACHE[key] = build_module(*key)
    return _MODULE_CACHE[key]


def kernel(s, x, gain, decay, noise):
    global LAST_RESULT
    from concourse.bass_utils import run_bass_kernel_spmd

    s = np.ascontiguousarray(np.asarray(s, dtype=np.float32))
    x = np.ascontiguousarray(np.asarray(x, dtype=np.float32))
    b, t = s.shape
    assert b % N_CORES == 0
    rows = b // N_CORES

    nc = _get_module((rows, t, float(gain), float(decay), float(noise)))

    in_maps = [
        {"s": s[i * rows:(i + 1) * rows], "x": x[i * rows:(i + 1) * rows]}
        for i in range(N_CORES)
    ]
    res = run_bass_kernel_spmd(nc, in_maps, list(range(N_CORES)))
    LAST_RESULT = res
    return np.concatenate([res.results[i]["out"] for i in range(N_CORES)],
                          axis=0)


# revision 6
# speedup vs baseline: 1.1830x; 1.1172x over previous
"""Trainium2 Bass kernel for nn_LogisticModel.

Computes, elementwise over [B, T] f32 inputs s, x:
    x_prev[:, t] = x[:, t-1]  (0 for t == 0)
    bias  = sigmoid(gain * s)
    resid = x - decay * x_prev - bias
    logp  = -0.5 * (resid / noise)^2 - (log(noise) + 0.5*log(2*pi))

Data-parallel over the batch axis: each of the 8 NeuronCores processes
B/8 = 512 rows. No cross-core communication (rows are independent).

Per-core schedule (memory-bound; HBM roofline ~48 MiB / ~360 GB/s ~ 140 us):
  - tiles of [128, W] columns; x is loaded as [128, W+1] with one extra
    leading column so both x and x_prev views come from a single DMA.
  - ACT (scalar engine): sigmoid, square(scale), final affine copy.
  - DVE (vector engine): (x_prev * -decay) + x, then subtract bias.
"""

import os
import sys
from contextlib import ExitStack

import numpy as np

for _p in ("/root/.axon_site", "/root/.axon_site/_ro/trn_rl_repo",
           "/root/.axon_site/_ro/pypackages", "/opt/trn_rl_repo"):
    if os.path.isdir(_p) and _p not in sys.path:
        sys.path.append(_p)

import concourse.bass as bass
import concourse.bacc as bacc
import concourse.mybir as mybir
import concourse.tile as tile

F32 = mybir.dt.float32
P = 128

N_CORES = 8
B, T = 4096, 8192

LAST_RESULT = None  # test harness introspection; unused by graders


def build_module(rows, cols, gain, decay, noise, W=4096, load_bufs=4,
                 work_bufs=3):
    """Build the single-core Bass module for a [rows, cols] shard."""
    assert rows % P == 0 and cols % W == 0
    nc = bacc.Bacc()
    s_in = nc.declare_dram_parameter("s", [rows, cols], F32, isOutput=False)
    x_in = nc.declare_dram_parameter("x", [rows, cols], F32, isOutput=False)
    out = nc.declare_dram_parameter("out", [rows, cols], F32, isOutput=True)

    log_norm = float(np.log(noise) + 0.5 * np.log(2.0 * np.pi))
    inv_noise = float(1.0 / noise)
    AF = mybir.ActivationFunctionType
    OP = mybir.AluOpType

    # Column-tile schedule per row-block.  The final row-block tapers off
    # into small tiles so the last (serial) compute chain + store after the
    # final load is short — it is pure DMA-idle tail time.
    def col_tiles(last_block):
        if not last_block or W <= 1024:
            return [W] * (cols // W)
        tiles, rem = [], cols
        while rem > W:
            tiles.append(W)
            rem -= W
        # taper: W -> W/2 -> ... -> 512, 512 (sums to W)
        t = W // 2
        while rem > 512:
            t = max(t, 512)
            tiles.append(t)
            rem -= t
            t //= 2
        tiles.append(rem)
        return tiles

    with tile.TileContext(nc) as tc, ExitStack() as ctx:
        loads = ctx.enter_context(tc.tile_pool(name="loads", bufs=load_bufs))
        work = ctx.enter_context(tc.tile_pool(name="work", bufs=work_bufs))
        n_rb = rows // P
        for rb in range(n_rb):
            r0 = rb * P
            c0 = 0
            for W_c in col_tiles(rb == n_rb - 1):
                # Loads on the SP HWDGE ring; stores on the ACT ring so
                # output stores don't head-of-line-block upcoming loads.
                s_t = loads.tile([P, W_c], F32, tag="s")
                nc.sync.dma_start(s_t[:], s_in[r0:r0 + P, c0:c0 + W_c])
                # x tile carries one extra leading column = x_prev source.
                # STT format (3 APs) only has room for ONE sync wait, so
                # x_t must have exactly one producer: for the first column
                # tile, load aligned and handle t=0 (x_prev = 0) with a
                # 1-column copy instead of a memset.
                x_t = loads.tile([P, W_c + 1], F32, tag="x")
                # bias = sigmoid(gain * s), in place over s
                nc.scalar.activation(s_t[:], s_t[:], AF.Sigmoid,
                                     scale=float(gain))
                t_t = work.tile([P, W_c], F32, tag="t")
                # t = x - decay * x_prev
                if c0 == 0:
                    nc.sync.dma_start(x_t[:, 0:W_c], x_in[r0:r0 + P, 0:W_c])
                    nc.vector.scalar_tensor_tensor(
                        t_t[:, 1:W_c], x_t[:, 0:W_c - 1], -float(decay),
                        x_t[:, 1:W_c], OP.mult, OP.add)
                    nc.vector.tensor_copy(t_t[:, 0:1], x_t[:, 0:1])
                else:
                    nc.sync.dma_start(x_t[:],
                                      x_in[r0:r0 + P, c0 - 1:c0 + W_c])
                    nc.vector.scalar_tensor_tensor(
                        t_t[:], x_t[:, 0:W_c], -float(decay),
                        x_t[:, 1:W_c + 1], OP.mult, OP.add)
                # resid = t - bias;  r2 = (resid/noise)^2;  out affine —
                # all in place over t_t.
                nc.vector.tensor_tensor(t_t[:], t_t[:], s_t[:], OP.subtract)
                nc.scalar.activation(t_t[:], t_t[:], AF.Square,
                                     scale=inv_noise)
                nc.scalar.activation(t_t[:], t_t[:], AF.Copy,
                                     bias=-log_norm, scale=-0.5)
                nc.scalar.dma_start(out[r0:r0 + P, c0:c0 + W_c], t_t[:])
                c0 += W_c
    # Bacc.compile() legalizes sync waits (TRN2: max 1 wait per instruction)
    nc.compile()
    return nc


_MODULE_CACHE = {}


def _get_module(key):
    if key not in _MODULE_CACHE:
        _MODULE_CACHE[key] = build_module(*key)
    return _MODULE_CACHE[key]


def kernel(s, x, gain, decay, noise):
    global LAST_RESULT
    from concourse.bass_utils import run_bass_kernel_spmd

    s = np.ascontiguousarray(np.asarray(s, dtype=np.float32))
    x = np.ascontiguousarray(np.asarray(x, dtype=np.float32))
    b, t = s.shape
    assert b % N_CORES == 0
    rows = b // N_CORES

    nc = _get_module((rows, t, float(gain), float(decay), float(noise)))

    in_maps = [
        {"s": s[i * rows:(i + 1) * rows], "x": x[i * rows:(i + 1) * rows]}
        for i in range(N_CORES)
    ]
    res = run_bass_kernel_spmd(nc, in_maps, list(range(N_CORES)))
    LAST_RESULT = res
    return np.concatenate([res.results[i]["out"] for i in range(N_CORES)],
                          axis=0)
